# revision 1
# baseline (speedup 1.0000x reference)
"""Trainium2 Bass kernel for nn_CMSWrite (hierarchical memory scatter-write).

Full inputs in, full output out. Internally shards the N=32768 memory slots
across 8 NeuronCores (4096 slots each). Per level:
  - tiny control nets (replicated on every core) produce gate g, value v[512],
    key k[128] from a 128-dim latent,
  - scores = K_mem @ k / sqrt(128) over the local slot shard,
  - softmax denominator via an 8-core AllReduce of the per-shard exp-sums
    (scores are in [-0.1, 0.1] so no max subtraction is needed),
  - decayed rank-1 write into M [N,512] and K [N,128], fully local per shard.

The heavy part is pure memory streaming: ~40MB read + 40MB write per core.
"""

import math
import numpy as np
from contextlib import ExitStack

# ---------------------------------------------------------------- constants
L = 4
N_FULL = 32768
N_CORES = 8
NSH = N_FULL // N_CORES          # 4096 slots per core per level
NCH = NSH // 128                 # 32 chunks of 128 slots
D_V = 512
D_K = 128
D_Z = 128
D_IN = 2560                      # padded control input (s | ctx | e)
N_IN_CH = D_IN // 128            # 20 contraction chunks
GM = 4                           # M chunks per DMA group (4*128 slots, 1MB)
GK = 8                           # K chunks per output DMA group
INV_SQRT_DK = 1.0 / math.sqrt(128.0)
EPS = 1e-5
THR = 0.1
N_COLS = 5                       # packed per-level columns: b1, wr, ln_g, ln_b, wg


def _ensure_path():
    try:
        import concourse  # noqa: F401
    except ImportError:
        import sys
        for p in ("/opt/trn_rl_repo", "/root/.axon_site/_ro/trn_rl_repo"):
            if p not in sys.path:
                sys.path.insert(0, p)


def _emit(ctx, tc, io, pools):
    """Emit one full update pass (all 4 levels)."""
    import concourse.bass as bass  # noqa: F401
    from concourse import mybir
    f32 = mybir.dt.float32
    Alu = mybir.AluOpType
    Act = mybir.ActivationFunctionType
    nc = tc.nc

    Msh, Ksh, W1T, Xs, Cols, WvT, WkT, Bv, Bk, Bg, Dec, Out = io
    const, small, big, psum, dram, m_in_p, m_out_p, k_out_p = pools

    ones_mat = const["ones_mat"]   # [128,128] of 1.0
    ones_col = const["ones_col"]   # [128,1]
    # brow: [128, 704] staging tile, zero everywhere except partition-0 row.
    # Broadcasting row r of brow to all partitions = ones_mat.T @ brow[:, r]
    # (the 127 zero rows contribute nothing). K=1 matmuls crash the HW, so
    # all partition-broadcasts go through this.
    brow = const["brow"]
    B_ROW2 = slice(0, 2)       # rstd, -mean*rstd
    B_CAT8 = slice(2, 10)      # coef[4], keep[4]
    B_KROW = slice(10, 138)    # k vector
    B_VROW = slice(138, 650)   # v vector

    # --- per-iteration small tiles -------------------------------------
    decay_row = small.tile([1, L], f32, tag="decay_row")
    nc.sync.dma_start(decay_row[:], Dec[:])
    keep_row = small.tile([1, L], f32, tag="keep_row")
    nc.scalar.activation(keep_row[:], decay_row[:], Act.Identity,
                         bias=1.0, scale=-1.0)
    bg_row = small.tile([1, L], f32, tag="bg_row")
    nc.sync.dma_start(bg_row[:], Bg[:])

    wgt_row = small.tile([1, L], f32, tag="wgt_row")     # gated g per level
    sums_row = small.tile([1, L], f32, tag="sums_row")   # local sum(exp) per level

    k_bs, v_bs, es_s, K_sbs = [], [], [], []

    # =================== phase A: control path + scores =================
    for ell in range(L):
        # ---- loads
        x_sb = small.tile([128, N_IN_CH], f32, tag="x_sb")
        nc.sync.dma_start(x_sb[:], Xs[ell].rearrange("(c p) -> p c", p=128))
        w1t_sb = big.tile([128, N_IN_CH, D_Z], f32, tag="w1t_sb")
        nc.sync.dma_start(w1t_sb[:], W1T[ell].rearrange("(c p) z -> p c z", p=128))
        cols_sb = small.tile([128, N_COLS], f32, tag="cols_sb")
        nc.sync.dma_start(cols_sb[:], Cols[ell].rearrange("c p -> p c"))
        wvt_sb = big.tile([128, D_V], f32, tag="wvt_sb")
        nc.sync.dma_start(wvt_sb[:], WvT[ell])
        wkt_sb = small.tile([128, D_K], f32, tag="wkt_sb")
        nc.sync.dma_start(wkt_sb[:], WkT[ell])
        bv_row = small.tile([1, D_V], f32, tag="bv_row")
        nc.sync.dma_start(bv_row[:], Bv[ell:ell + 1, :])
        bk_row = small.tile([1, D_K], f32, tag="bk_row")
        nc.sync.dma_start(bk_row[:], Bk[ell:ell + 1, :])

        b1_col = cols_sb[:, 0:1]
        wr_col = cols_sb[:, 1:2]
        lng_col = cols_sb[:, 2:3]
        lnb_col = cols_sb[:, 3:4]
        wg_col = cols_sb[:, 4:5]

        # ---- h = W1 @ x  (as column on partitions), y = (h + b1) * wr
        ph = psum.tile([128, 1], f32, tag="ph")
        for c in range(N_IN_CH):
            nc.tensor.matmul(ph[:], w1t_sb[:, c, :], x_sb[:, c:c + 1],
                             start=(c == 0), stop=(c == N_IN_CH - 1))
        y = small.tile([128, 1], f32, tag="y")
        nc.vector.scalar_tensor_tensor(y[:], ph[:], b1_col, wr_col,
                                       op0=Alu.add, op1=Alu.mult)

        # ---- layernorm stats via PE partition-reduce
        pstat = psum.tile([1, 2], f32, tag="pstat")
        nc.tensor.matmul(pstat[:, 0:1], y[:], ones_col[:], start=True, stop=True)
        nc.tensor.matmul(pstat[:, 1:2], y[:], y[:], start=True, stop=True)
        mean = small.tile([1, 1], f32, tag="mean")
        nc.vector.tensor_scalar_mul(mean[:], pstat[:, 0:1], 1.0 / 128.0)
        var = small.tile([1, 1], f32, tag="var")
        # var = E[y^2] - mean^2 = (sumsq/128) - mean*mean
        msq = small.tile([1, 1], f32, tag="msq")
        nc.vector.tensor_mul(msq[:], mean[:], mean[:])
        nc.vector.scalar_tensor_tensor(var[:], pstat[:, 1:2], 1.0 / 128.0, msq[:],
                                       op0=Alu.mult, op1=Alu.subtract)
        sd = small.tile([1, 1], f32, tag="sd")
        nc.scalar.activation(sd[:], var[:], Act.Sqrt, bias=const["eps_cell"][:])
        row2 = brow[0:1, B_ROW2]
        nc.vector.reciprocal(row2[:, 0:1], sd[:])                  # rstd
        nc.vector.scalar_tensor_tensor(row2[:, 1:2], mean[:], -1.0, row2[:, 0:1],
                                       op0=Alu.mult, op1=Alu.mult)  # -mean*rstd
        pbc = psum.tile([128, 2], f32, tag="pbc")
        nc.tensor.matmul(pbc[:], ones_mat[:], brow[:, B_ROW2], start=True, stop=True)
        bc2 = small.tile([128, 2], f32, tag="bc2")
        nc.scalar.copy(bc2[:], pbc[:])

        # ---- z = ((y - mean) * rstd) * ln_g + ln_b
        z0 = small.tile([128, 1], f32, tag="z0")
        nc.scalar.activation(z0[:], y[:], Act.Identity,
                             bias=bc2[:, 1:2], scale=bc2[:, 0:1])
        z = small.tile([128, 1], f32, tag="z")
        nc.vector.scalar_tensor_tensor(z[:], z0[:], lng_col, lnb_col,
                                       op0=Alu.mult, op1=Alu.add)

        # ---- gate g, value v, key k
        pg = psum.tile([1, 1], f32, tag="pstat")
        nc.tensor.matmul(pg[:], z[:], wg_col, start=True, stop=True)
        g = small.tile([1, 1], f32, tag="g")
        nc.scalar.activation(g[:], pg[:], Act.Sigmoid,
                             bias=bg_row[:, ell:ell + 1], scale=1.0)
        mask = small.tile([1, 1], f32, tag="mask")
        nc.vector.tensor_scalar(mask[:], g[:], THR, None, Alu.is_ge)
        nc.vector.tensor_mul(wgt_row[:, ell:ell + 1], g[:], mask[:])

        pv = psum.tile([1, D_V], f32, tag="pv")
        nc.tensor.matmul(pv[:], z[:], wvt_sb[:], start=True, stop=True)
        vpre = small.tile([1, D_V], f32, tag="vpre")
        nc.vector.tensor_add(vpre[:], pv[:], bv_row[:])
        v_row = brow[0:1, B_VROW]
        nc.scalar.activation(v_row[:], vpre[:], Act.Tanh)

        pk = psum.tile([1, D_K], f32, tag="pk")
        nc.tensor.matmul(pk[:], z[:], wkt_sb[:], start=True, stop=True)
        k_row = brow[0:1, B_KROW]
        nc.vector.tensor_add(k_row[:], pk[:], bk_row[:])

        # ---- broadcast k and v across partitions (zero-padded ones matmul)
        pkb = psum.tile([128, D_K], f32, tag="pkb")
        nc.tensor.matmul(pkb[:], ones_mat[:], brow[:, B_KROW], start=True, stop=True)
        k_b = big.tile([128, D_K], f32, tag=f"k_b{ell}")
        nc.scalar.copy(k_b[:], pkb[:])
        pvb = psum.tile([128, D_V], f32, tag="pvb")
        nc.tensor.matmul(pvb[:], ones_mat[:], brow[:, B_VROW], start=True, stop=True)
        v_b = big.tile([128, D_V], f32, tag=f"v_b{ell}")
        nc.scalar.copy(v_b[:], pvb[:])

        # ---- K shard load (kept resident for the update phase)
        K_sb = big.tile([128, NCH, D_K], f32, tag=f"K_sb{ell}")
        for h in range(2):
            nc.sync.dma_start(
                K_sb[:, h * (NCH // 2):(h + 1) * (NCH // 2), :],
                Ksh[ell, h * (NSH // 2):(h + 1) * (NSH // 2), :]
                .rearrange("(c p) f -> p c f", p=128))

        # ---- scores and exp: (K * 1/sqrt(dk)) * k_b, row-accumulated
        scores = small.tile([128, NCH], f32, tag="scores")
        for c in range(NCH):
            scratch = small.tile([128, D_K], f32, tag="scratch")
            nc.vector.scalar_tensor_tensor(
                scratch[:], K_sb[:, c, :], INV_SQRT_DK, k_b[:],
                op0=Alu.mult, op1=Alu.mult,
                accum_out=scores[:, c:c + 1])
        es = big.tile([128, NCH], f32, tag=f"es{ell}")
        rowsum = small.tile([128, 1], f32, tag="rowsum")
        nc.scalar.activation(es[:], scores[:], Act.Exp, accum_out=rowsum[:])
        ps = psum.tile([1, 1], f32, tag="pstat")
        nc.tensor.matmul(ps[:], rowsum[:], ones_col[:], start=True, stop=True)
        nc.scalar.copy(sums_row[:, ell:ell + 1], ps[:])

        k_bs.append(k_b); v_bs.append(v_b); es_s.append(es); K_sbs.append(K_sb)

    # =================== AllReduce of the 4 exp-sums ====================
    cc_in = dram.tile([1, L], f32, tag="cc_in")
    cc_out = dram.tile([1, L], f32, tag="cc_out", addr_space="Shared")
    nc.gpsimd.dma_start(cc_in[:], sums_row[:])
    nc.gpsimd.collective_compute(
        "AllReduce", Alu.add,
        replica_groups=[list(range(N_CORES))],
        ins=[cc_in[:].opt()], outs=[cc_out[:].opt()])
    denom_row = small.tile([1, L], f32, tag="denom_row")
    nc.gpsimd.dma_start(denom_row[:], cc_out[:])

    # coef = wgt / denom ; broadcast [coef | keep] to all partitions
    rcp_row = small.tile([1, L], f32, tag="rcp_row")
    nc.vector.reciprocal(rcp_row[:], denom_row[:])
    cat_row = brow[0:1, B_CAT8]
    nc.vector.tensor_mul(cat_row[:, 0:L], wgt_row[:], rcp_row[:])
    nc.vector.tensor_copy(cat_row[:, L:2 * L], keep_row[:])
    pbc8 = psum.tile([128, 2 * L], f32, tag="pbc")
    nc.tensor.matmul(pbc8[:], ones_mat[:], brow[:, B_CAT8], start=True, stop=True)
    bc8 = small.tile([128, 2 * L], f32, tag="bc8")
    nc.scalar.copy(bc8[:], pbc8[:])

    # =================== phase B: streamed rank-1 updates ===============
    for ell in range(L):
        keep_col = bc8[:, L + ell:L + ell + 1]
        w_tile = big.tile([128, NCH], f32, tag=f"w_tile{ell}")
        nc.vector.tensor_scalar_mul(w_tile[:], es_s[ell][:], bc8[:, ell:ell + 1])

        # M stream: out = keep * M + w ⊗ v
        for gidx in range(NCH // GM):
            m_in = m_in_p.tile([128, GM, D_V], f32, tag="m_in")
            nc.sync.dma_start(
                m_in[:],
                Msh[ell, gidx * GM * 128:(gidx + 1) * GM * 128, :]
                .rearrange("(c p) f -> p c f", p=128))
            m_out = m_out_p.tile([128, GM, D_V], f32, tag="m_out")
            for s in range(GM):
                c = gidx * GM + s
                t1 = small.tile([128, D_V], f32, tag="t1")
                nc.scalar.activation(t1[:], v_bs[ell][:], Act.Copy,
                                     scale=w_tile[:, c:c + 1])
                nc.vector.scalar_tensor_tensor(
                    m_out[:, s, :], m_in[:, s, :], keep_col, t1[:],
                    op0=Alu.mult, op1=Alu.add)
            nc.sync.dma_start(
                Out[ell, gidx * GM * 128:(gidx + 1) * GM * 128, 0:D_V]
                .rearrange("(c p) f -> p c f", p=128),
                m_out[:])

        # K stream (shard already in SBUF): out = keep * K + w ⊗ k
        for gidx in range(NCH // GK):
            k_out = k_out_p.tile([128, GK, D_K], f32, tag="k_out")
            for s in range(GK):
                c = gidx * GK + s
                t1k = small.tile([128, D_K], f32, tag="t1k")
                nc.scalar.activation(t1k[:], k_bs[ell][:], Act.Copy,
                                     scale=w_tile[:, c:c + 1])
                nc.vector.scalar_tensor_tensor(
                    k_out[:, s, :], K_sbs[ell][:, c, :], keep_col, t1k[:],
                    op0=Alu.mult, op1=Alu.add)
            nc.sync.dma_start(
                Out[ell, gidx * GK * 128:(gidx + 1) * GK * 128, D_V:D_V + D_K]
                .rearrange("(c p) f -> p c f", p=128),
                k_out[:])


def build(iters=1):
    """Build + compile the Bass program. Returns the nc object."""
    _ensure_path()
    import concourse.bacc as bacc
    import concourse.tile as tile
    from concourse import mybir
    f32 = mybir.dt.float32

    nc = bacc.Bacc("TRN2", target_bir_lowering=False, debug=False,
                   enable_asserts=True, num_devices=N_CORES)

    io = (
        nc.dram_tensor("m_sh", [L, NSH, D_V], f32, kind="ExternalInput").ap(),
        nc.dram_tensor("k_sh", [L, NSH, D_K], f32, kind="ExternalInput").ap(),
        nc.dram_tensor("w1t", [L, D_IN, D_Z], f32, kind="ExternalInput").ap(),
        nc.dram_tensor("xs", [L, D_IN], f32, kind="ExternalInput").ap(),
        nc.dram_tensor("cols", [L, N_COLS, D_Z], f32, kind="ExternalInput").ap(),
        nc.dram_tensor("wvt", [L, D_Z, D_V], f32, kind="ExternalInput").ap(),
        nc.dram_tensor("wkt", [L, D_Z, D_K], f32, kind="ExternalInput").ap(),
        nc.dram_tensor("bv", [L, D_V], f32, kind="ExternalInput").ap(),
        nc.dram_tensor("bk", [L, D_K], f32, kind="ExternalInput").ap(),
        nc.dram_tensor("bg", [1, L], f32, kind="ExternalInput").ap(),
        nc.dram_tensor("decay", [1, L], f32, kind="ExternalInput").ap(),
        nc.dram_tensor("out", [L, NSH, D_V + D_K], f32, kind="ExternalOutput").ap(),
    )

    with tile.TileContext(nc) as tc, ExitStack() as ctx:
        const_p = ctx.enter_context(tc.tile_pool(name="const", bufs=1))
        small = ctx.enter_context(tc.tile_pool(name="small", bufs=2))
        big = ctx.enter_context(tc.tile_pool(name="big", bufs=1))
        psum = ctx.enter_context(tc.tile_pool(name="psum", bufs=1, space="PSUM"))
        dram = ctx.enter_context(tc.tile_pool(name="dram", bufs=2, space="DRAM"))
        m_in_p = ctx.enter_context(tc.tile_pool(name="m_in_p", bufs=4))
        m_out_p = ctx.enter_context(tc.tile_pool(name="m_out_p", bufs=3))
        k_out_p = ctx.enter_context(tc.tile_pool(name="k_out_p", bufs=2))

        ones_mat = const_p.tile([128, 128], f32)
        nc.vector.memset(ones_mat[:], 1.0)
        ones_col = const_p.tile([128, 1], f32)
        nc.vector.memset(ones_col[:], 1.0)
        eps_cell = const_p.tile([1, 1], f32)
        nc.vector.memset(eps_cell[:], EPS)
        brow = const_p.tile([128, 704], f32)
        nc.vector.memset(brow[:], 0.0)
        const = {"ones_mat": ones_mat, "ones_col": ones_col,
                 "eps_cell": eps_cell, "brow": brow}

        pools = (const, small, big, psum, dram, m_in_p, m_out_p, k_out_p)
        for _ in range(iters):
            _emit(ctx, tc, io, pools)

    nc.compile()
    return nc


def marshal(inputs):
    """Host-side input marshalling: shard M/K, pre-transpose tiny weights."""
    f = lambda a: np.ascontiguousarray(np.asarray(a, dtype=np.float32))
    s_t, e_t = f(inputs["s_t"]), f(inputs["e_t"])
    ctxs = f(inputs["level_contexts"])
    M, K_mem = f(inputs["M"]), f(inputs["K_mem"])
    W1_0, b1_0 = f(inputs["W1_0"]), f(inputs["b1_0"])
    W1_r, b1_r = f(inputs["W1_r"]), f(inputs["b1_r"])

    xs = np.zeros((L, D_IN), np.float32)
    w1t = np.zeros((L, D_IN, D_Z), np.float32)
    xs[0, 0:1024] = s_t
    xs[0, 1536:2560] = e_t
    w1t[0, 0:1024] = W1_0[:, 0:1024].T
    w1t[0, 1536:2560] = W1_0[:, 1024:2048].T
    for ell in range(1, L):
        xs[ell] = np.concatenate([s_t, ctxs[ell - 1], e_t])
        w1t[ell] = W1_r[ell - 1].T

    cols = np.zeros((L, N_COLS, D_Z), np.float32)
    for ell in range(L):
        cols[ell, 0] = b1_0 if ell == 0 else b1_r[ell - 1]
        cols[ell, 1] = f(inputs["spec_wr"])[ell, 0]
        cols[ell, 2] = f(inputs["ln_g"])[ell]
        cols[ell, 3] = f(inputs["ln_b"])[ell]
        cols[ell, 4] = f(inputs["Wg"])[ell, 0]

    common = {
        "w1t": w1t, "xs": xs, "cols": cols,
        "wvt": np.ascontiguousarray(f(inputs["Wv"]).transpose(0, 2, 1)),
        "wkt": np.ascontiguousarray(f(inputs["Wk"]).transpose(0, 2, 1)),
        "bv": f(inputs["bv"]), "bk": f(inputs["bk"]),
        "bg": f(inputs["bg"]).reshape(1, L),
        "decay": f(inputs["decay"]).reshape(1, L),
    }
    in_maps = []
    for c in range(N_CORES):
        sl = slice(c * NSH, (c + 1) * NSH)
        in_maps.append(dict(common,
                            m_sh=np.ascontiguousarray(M[:, sl, :]),
                            k_sh=np.ascontiguousarray(K_mem[:, sl, :])))
    return in_maps


_BUILD_CACHE = {}


def kernel(**inputs):
    _ensure_path()
    from concourse import bass_utils

    if 1 not in _BUILD_CACHE:
        _BUILD_CACHE[1] = build(iters=1)
    nc = _BUILD_CACHE[1]

    in_maps = marshal(inputs)
    r = bass_utils.run_bass_kernel_spmd(nc, in_maps,
                                        core_ids=list(range(N_CORES)))
    full = np.empty((L, N_FULL, D_V + D_K), np.float32)
    for c in range(N_CORES):
        full[:, c * NSH:(c + 1) * NSH, :] = r.results[c]["out"]
    return full



# revision 2
# speedup vs baseline: 3.8506x; 3.8506x over previous
"""Trainium2 Bass kernel for nn_CMSWrite (hierarchical memory scatter-write).

Full inputs in, full output out. Internally shards the N=32768 memory slots
across 8 NeuronCores (4096 slots each). Per level:
  - tiny control nets (replicated on every core) produce gate g, value v[512],
    key k[128] from a 128-dim latent,
  - scores = K_mem @ k / sqrt(128) over the local slot shard,
  - softmax denominator via a per-level 8-core AllReduce of the local
    exp-sums, software-pipelined so each collective flies while the previous
    level's memory stream saturates DMA,
  - decayed rank-1 write into M [N,512] and K [N,128], fully local per shard.

Key layout/throughput choices (all measured):
  - Host marshals the slot shard partition-major ([128 partitions, 32
    chunks]) so every DMA descriptor is a 16-20KB fully-contiguous run.
  - M and K output columns are fused into one [128, G, 640] tile per group
    so stores are single wide DMAs.
  - The per-level decay scale `keep` is folded out of the device stream
    (device computes Out = M + (wgt*alpha/keep) x v; the host epilogue
    multiplies by keep). One fused DVE op per 128x512 chunk.
  - All streaming DMAs issue on the sync-engine HWDGE ring: a single FIFO
    ring coarsely alternates 8MB read / 10MB write bursts, which measures
    faster than splitting loads/stores across rings.
  - Per level, all 4 M-group loads are issued before the first store so a
    store's compute wait cannot block load issue (in-order ring).

The heavy part is pure memory streaming: ~42MB read + 42MB write per core.
"""

import math
import numpy as np
from contextlib import ExitStack

# ---------------------------------------------------------------- constants
L = 4
N_FULL = 32768
N_CORES = 8
NSH = N_FULL // N_CORES          # 4096 slots per core per level
NCH = NSH // 128                 # 32 chunks of 128 slots
D_V = 512
D_K = 128
D_Z = 128
D_O = D_V + D_K                  # 640 fused output row
D_IN = 2560                      # padded control input (s | ctx | e)
N_IN_CH = D_IN // 128            # 20 contraction chunks
GM = 8                           # chunks per M-stream group (2MB loads)
NG = NCH // GM                   # 4 groups per level
INV_SQRT_DK = 1.0 / math.sqrt(128.0)
EPS = 1e-5
THR = 0.1
N_COLS = 5                       # packed per-level columns: b1, wr, ln_g, ln_b, wg


def _ensure_path():
    try:
        import concourse  # noqa: F401
    except ImportError:
        import sys
        for p in ("/opt/trn_rl_repo", "/root/.axon_site/_ro/trn_rl_repo"):
            if p not in sys.path:
                sys.path.insert(0, p)


def _emit(tc, io, pools):
    """Emit one full update pass, software-pipelined A0 A1 B0 A2 B1 A3 B2 B3
    so each level's AllReduce is in flight a full streaming phase before its
    result gates the stores."""
    from concourse import mybir
    f32 = mybir.dt.float32
    Alu = mybir.AluOpType
    Act = mybir.ActivationFunctionType
    nc = tc.nc

    Msh, Ksh, W1T, Xs, Cols, WvT, WkT, Bv, Bk, Bg, Dec, Out = io
    const, small, wpool, kpool, espool, psum, dram, m_in_p, out_p = pools

    ones_mat = const["ones_mat"]   # [128,128] of 1.0
    ones_col = const["ones_col"]   # [128,1]
    # brow: [128, 704] staging tile, zero everywhere except partition-0 row.
    # Broadcasting row r of brow to all partitions = ones_mat.T @ brow[:, r]
    # (the 127 zero rows contribute nothing). K=1 matmuls crash the HW, so
    # all partition-broadcasts go through this.
    brow = const["brow"]
    B_ROW2 = slice(0, 2)       # rstd, -mean*rstd
    B_CK1 = slice(2, 3)        # coef
    B_KROW = slice(10, 138)    # k vector
    B_VROW = slice(138, 650)   # v vector

    # --- per-iteration small tiles -------------------------------------
    decay_row = small.tile([1, L], f32, tag="decay_row")
    nc.sync.dma_start(decay_row[:], Dec[:])
    keep_row = small.tile([1, L], f32, tag="keep_row")
    nc.scalar.activation(keep_row[:], decay_row[:], Act.Identity,
                         bias=1.0, scale=-1.0)
    bg_row = small.tile([1, L], f32, tag="bg_row")
    nc.sync.dma_start(bg_row[:], Bg[:])

    def emit_A(ell):
        # ================= phase A(ell): control path + scores ==========
        x_sb = small.tile([128, N_IN_CH], f32, tag="x_sb")
        nc.sync.dma_start(x_sb[:], Xs[ell])
        w1t_sb = wpool.tile([128, N_IN_CH, D_Z], f32, tag="w1t_sb")
        nc.sync.dma_start(w1t_sb[:], W1T[ell])
        cols_sb = small.tile([128, N_COLS], f32, tag="cols_sb")
        nc.sync.dma_start(cols_sb[:], Cols[ell])
        wvt_sb = wpool.tile([128, D_V], f32, tag="wvt_sb")
        nc.sync.dma_start(wvt_sb[:], WvT[ell])
        wkt_sb = wpool.tile([128, D_K], f32, tag="wkt_sb")
        nc.sync.dma_start(wkt_sb[:], WkT[ell])
        bv_row = small.tile([1, D_V], f32, tag="bv_row")
        nc.sync.dma_start(bv_row[:], Bv[ell:ell + 1, :])
        bk_row = small.tile([1, D_K], f32, tag="bk_row")
        nc.sync.dma_start(bk_row[:], Bk[ell:ell + 1, :])

        b1_col = cols_sb[:, 0:1]
        wr_col = cols_sb[:, 1:2]
        lng_col = cols_sb[:, 2:3]
        lnb_col = cols_sb[:, 3:4]
        wg_col = cols_sb[:, 4:5]

        # ---- h = W1 @ x  (as column on partitions), y = (h + b1) * wr
        ph = psum.tile([128, 1], f32, tag="ph")
        for c in range(N_IN_CH):
            nc.tensor.matmul(ph[:], w1t_sb[:, c, :], x_sb[:, c:c + 1],
                             start=(c == 0), stop=(c == N_IN_CH - 1))
        y = small.tile([128, 1], f32, tag="y")
        nc.vector.scalar_tensor_tensor(y[:], ph[:], b1_col, wr_col,
                                       op0=Alu.add, op1=Alu.mult)

        # ---- layernorm stats via PE partition-reduce
        pstat = psum.tile([1, 2], f32, tag="pstat")
        nc.tensor.matmul(pstat[:, 0:1], y[:], ones_col[:], start=True, stop=True)
        nc.tensor.matmul(pstat[:, 1:2], y[:], y[:], start=True, stop=True)
        mean = small.tile([1, 1], f32, tag="mean")
        nc.vector.tensor_scalar_mul(mean[:], pstat[:, 0:1], 1.0 / 128.0)
        var = small.tile([1, 1], f32, tag="var")
        # var = E[y^2] - mean^2 = (sumsq/128) - mean*mean
        msq = small.tile([1, 1], f32, tag="msq")
        nc.vector.tensor_mul(msq[:], mean[:], mean[:])
        nc.vector.scalar_tensor_tensor(var[:], pstat[:, 1:2], 1.0 / 128.0, msq[:],
                                       op0=Alu.mult, op1=Alu.subtract)
        sd = small.tile([1, 1], f32, tag="sd")
        nc.scalar.activation(sd[:], var[:], Act.Sqrt, bias=const["eps_cell"][:])
        row2 = brow[0:1, B_ROW2]
        nc.vector.reciprocal(row2[:, 0:1], sd[:])                  # rstd
        nc.vector.scalar_tensor_tensor(row2[:, 1:2], mean[:], -1.0, row2[:, 0:1],
                                       op0=Alu.mult, op1=Alu.mult)  # -mean*rstd
        pbc = psum.tile([128, 2], f32, tag="pbc")
        nc.tensor.matmul(pbc[:], ones_mat[:], brow[:, B_ROW2], start=True, stop=True)
        bc2 = small.tile([128, 2], f32, tag="bc2")
        nc.scalar.copy(bc2[:], pbc[:])

        # ---- z = ((y - mean) * rstd) * ln_g + ln_b
        z0 = small.tile([128, 1], f32, tag="z0")
        nc.scalar.activation(z0[:], y[:], Act.Identity,
                             bias=bc2[:, 1:2], scale=bc2[:, 0:1])
        z = small.tile([128, 1], f32, tag="z")
        nc.vector.scalar_tensor_tensor(z[:], z0[:], lng_col, lnb_col,
                                       op0=Alu.mult, op1=Alu.add)

        # ---- gate g, value v, key k
        pg = psum.tile([1, 1], f32, tag="pstat")
        nc.tensor.matmul(pg[:], z[:], wg_col, start=True, stop=True)
        g = small.tile([1, 1], f32, tag="g")
        nc.scalar.activation(g[:], pg[:], Act.Sigmoid,
                             bias=bg_row[:, ell:ell + 1], scale=1.0)
        mask = small.tile([1, 1], f32, tag="mask")
        nc.vector.tensor_scalar(mask[:], g[:], THR, None, Alu.is_ge)
        wgt = small.tile([1, 1], f32, tag="wgt")
        nc.vector.tensor_mul(wgt[:], g[:], mask[:])

        pv = psum.tile([1, D_V], f32, tag="pv")
        nc.tensor.matmul(pv[:], z[:], wvt_sb[:], start=True, stop=True)
        vpre = small.tile([1, D_V], f32, tag="vpre")
        nc.vector.tensor_add(vpre[:], pv[:], bv_row[:])
        v_row = brow[0:1, B_VROW]
        nc.scalar.activation(v_row[:], vpre[:], Act.Tanh)

        pk = psum.tile([1, D_K], f32, tag="pk")
        nc.tensor.matmul(pk[:], z[:], wkt_sb[:], start=True, stop=True)
        k_row = brow[0:1, B_KROW]
        nc.vector.tensor_add(k_row[:], pk[:], bk_row[:])

        # ---- broadcast k and v across partitions (zero-padded ones matmul)
        pkb = psum.tile([128, D_K], f32, tag="pkb")
        nc.tensor.matmul(pkb[:], ones_mat[:], brow[:, B_KROW], start=True, stop=True)
        k_b = espool.tile([128, D_K], f32, tag="k_b")
        nc.scalar.copy(k_b[:], pkb[:])
        pvb = psum.tile([128, D_V], f32, tag="pvb")
        nc.tensor.matmul(pvb[:], ones_mat[:], brow[:, B_VROW], start=True, stop=True)
        v_b = espool.tile([128, D_V], f32, tag="v_b")
        nc.scalar.copy(v_b[:], pvb[:])

        # ---- K shard load (kept resident for the update phase)
        K_sb = kpool.tile([128, NCH, D_K], f32, tag="K_sb")
        nc.sync.dma_start(K_sb[:], Ksh[ell])

        # ---- scores and exp: (K * 1/sqrt(dk)) * k_b, row-accumulated
        scores = small.tile([128, NCH], f32, tag="scores")
        for c in range(NCH):
            scratch = small.tile([128, D_K], f32, tag="scratch")
            nc.vector.scalar_tensor_tensor(
                scratch[:], K_sb[:, c, :], INV_SQRT_DK, k_b[:],
                op0=Alu.mult, op1=Alu.mult,
                accum_out=scores[:, c:c + 1])
        es = espool.tile([128, NCH], f32, tag="es")
        rowsum = small.tile([128, 1], f32, tag="rowsum")
        nc.scalar.activation(es[:], scores[:], Act.Exp, accum_out=rowsum[:])
        ps = psum.tile([1, 1], f32, tag="pstat")
        nc.tensor.matmul(ps[:], rowsum[:], ones_col[:], start=True, stop=True)
        lsum = small.tile([1, 1], f32, tag="lsum")
        nc.scalar.copy(lsum[:], ps[:])

        # ---- AllReduce of this level's exp-sum (pipelines behind B(ell-1))
        cc_in = dram.tile([1, 1], f32, tag="cc_in")
        cc_out = dram.tile([1, 1], f32, tag="cc_out", addr_space="Shared")
        nc.scalar.dma_start(cc_in[:], lsum[:])
        nc.gpsimd.collective_compute(
            "AllReduce", Alu.add,
            replica_groups=[list(range(N_CORES))],
            ins=[cc_in[:].opt()], outs=[cc_out[:].opt()])
        denom = small.tile([1, 1], f32, tag="denom")
        nc.scalar.dma_start(denom[:], cc_out[:])
        return dict(wgt=wgt, k_b=k_b, v_b=v_b, K_sb=K_sb, es=es, denom=denom)

    def emit_B(ell, st):
        # ================= phase B(ell): streamed rank-1 update =========
        wgt, k_b, v_b, K_sb, es, denom = (
            st["wgt"], st["k_b"], st["v_b"], st["K_sb"], st["es"], st["denom"])
        # coef = wgt / (denom * keep); the host multiplies the returned
        # output by keep, so the device stream skips the keep*M scale.
        dk = small.tile([1, 1], f32, tag="dk")
        nc.vector.tensor_mul(dk[:], denom[:], keep_row[:, ell:ell + 1])
        rcp = small.tile([1, 1], f32, tag="rcp")
        nc.vector.reciprocal(rcp[:], dk[:])
        ck_row = brow[0:1, B_CK1]
        nc.vector.tensor_mul(ck_row[:], wgt[:], rcp[:])
        pck = psum.tile([128, 1], f32, tag="pbc")
        nc.tensor.matmul(pck[:], ones_mat[:], brow[:, B_CK1], start=True, stop=True)
        coef_col = small.tile([128, 1], f32, tag="coef_col")
        nc.scalar.copy(coef_col[:], pck[:])

        w_tile = espool.tile([128, NCH], f32, tag="w_tile")
        nc.vector.tensor_scalar_mul(w_tile[:], es[:], coef_col[:])

        # fused M|K stream: ot[:, 0:512] = M + w' x v, ot[:, 512:640] = K + w' x k
        # All loads issue before the first store so a store's compute wait
        # cannot block load issue on the in-order sync ring.
        m_ins = []
        for g in range(NG):
            m_in_g = m_in_p.tile([128, GM, D_V], f32, tag="m_in")
            nc.sync.dma_start(m_in_g[:], Msh[ell, :, g * GM:(g + 1) * GM, :])
            m_ins.append(m_in_g)
        for g in range(NG):
            ot = out_p.tile([128, GM, D_O], f32, tag="ot")
            for s in range(GM):
                c = g * GM + s
                nc.vector.scalar_tensor_tensor(
                    ot[:, s, 0:D_V], v_b[:], w_tile[:, c:c + 1],
                    m_ins[g][:, s, :], op0=Alu.mult, op1=Alu.add)
                nc.vector.scalar_tensor_tensor(
                    ot[:, s, D_V:D_O], k_b[:], w_tile[:, c:c + 1],
                    K_sb[:, c, :], op0=Alu.mult, op1=Alu.add)
            nc.sync.dma_start(Out[ell, :, g * GM:(g + 1) * GM, :], ot[:])

    # software pipeline: A0 A1 B0 A2 B1 A3 B2 B3
    st = {0: emit_A(0)}
    for ell in range(L):
        if ell + 1 < L:
            st[ell + 1] = emit_A(ell + 1)
        emit_B(ell, st.pop(ell))


def build(iters=1):
    """Build + compile the Bass program. Returns the nc object."""
    _ensure_path()
    import concourse.bacc as bacc
    import concourse.tile as tile
    from concourse import mybir
    f32 = mybir.dt.float32

    nc = bacc.Bacc("TRN2", target_bir_lowering=False, debug=False,
                   enable_asserts=True, num_devices=N_CORES)

    io = (
        nc.dram_tensor("m_sh", [L, 128, NCH, D_V], f32, kind="ExternalInput").ap(),
        nc.dram_tensor("k_sh", [L, 128, NCH, D_K], f32, kind="ExternalInput").ap(),
        nc.dram_tensor("w1t", [L, 128, N_IN_CH, D_Z], f32, kind="ExternalInput").ap(),
        nc.dram_tensor("xs", [L, 128, N_IN_CH], f32, kind="ExternalInput").ap(),
        nc.dram_tensor("cols", [L, 128, N_COLS], f32, kind="ExternalInput").ap(),
        nc.dram_tensor("wvt", [L, D_Z, D_V], f32, kind="ExternalInput").ap(),
        nc.dram_tensor("wkt", [L, D_Z, D_K], f32, kind="ExternalInput").ap(),
        nc.dram_tensor("bv", [L, D_V], f32, kind="ExternalInput").ap(),
        nc.dram_tensor("bk", [L, D_K], f32, kind="ExternalInput").ap(),
        nc.dram_tensor("bg", [1, L], f32, kind="ExternalInput").ap(),
        nc.dram_tensor("decay", [1, L], f32, kind="ExternalInput").ap(),
        nc.dram_tensor("out", [L, 128, NCH, D_O], f32, kind="ExternalOutput").ap(),
    )

    with tile.TileContext(nc) as tc, ExitStack() as ctx:
        const_p = ctx.enter_context(tc.tile_pool(name="const", bufs=1))
        small = ctx.enter_context(tc.tile_pool(name="small", bufs=2))
        wpool = ctx.enter_context(tc.tile_pool(name="wpool", bufs=2))
        kpool = ctx.enter_context(tc.tile_pool(name="kpool", bufs=2))
        espool = ctx.enter_context(tc.tile_pool(name="espool", bufs=2))
        psum = ctx.enter_context(tc.tile_pool(name="psum", bufs=1, space="PSUM"))
        dram = ctx.enter_context(tc.tile_pool(name="dram", bufs=4, space="DRAM"))
        m_in_p = ctx.enter_context(tc.tile_pool(name="m_in_p", bufs=4))
        out_p = ctx.enter_context(tc.tile_pool(name="out_p", bufs=3))

        ones_mat = const_p.tile([128, 128], f32)
        nc.vector.memset(ones_mat[:], 1.0)
        ones_col = const_p.tile([128, 1], f32)
        nc.vector.memset(ones_col[:], 1.0)
        eps_cell = const_p.tile([1, 1], f32)
        nc.vector.memset(eps_cell[:], EPS)
        brow = const_p.tile([128, 704], f32)
        nc.vector.memset(brow[:], 0.0)
        const = {"ones_mat": ones_mat, "ones_col": ones_col,
                 "eps_cell": eps_cell, "brow": brow}

        pools = (const, small, wpool, kpool, espool, psum, dram, m_in_p, out_p)
        for _ in range(iters):
            _emit(tc, io, pools)

    nc.compile()
    return nc


def marshal(inputs):
    """Host-side input marshalling: shard M/K partition-major, pre-transpose
    the tiny control-net weights so every device DMA is contiguous."""
    f = lambda a: np.ascontiguousarray(np.asarray(a, dtype=np.float32))
    s_t, e_t = f(inputs["s_t"]), f(inputs["e_t"])
    ctxs = f(inputs["level_contexts"])
    M, K_mem = f(inputs["M"]), f(inputs["K_mem"])
    W1_0, b1_0 = f(inputs["W1_0"]), f(inputs["b1_0"])
    W1_r, b1_r = f(inputs["W1_r"]), f(inputs["b1_r"])

    xs = np.zeros((L, D_IN), np.float32)
    w1t = np.zeros((L, D_IN, D_Z), np.float32)
    xs[0, 0:1024] = s_t
    xs[0, 1536:2560] = e_t
    w1t[0, 0:1024] = W1_0[:, 0:1024].T
    w1t[0, 1536:2560] = W1_0[:, 1024:2048].T
    for ell in range(1, L):
        xs[ell] = np.concatenate([s_t, ctxs[ell - 1], e_t])
        w1t[ell] = W1_r[ell - 1].T
    # partition-major: element i = c*128 + p  ->  [p, c]
    xs_pc = np.ascontiguousarray(xs.reshape(L, N_IN_CH, 128).transpose(0, 2, 1))
    w1t_pc = np.ascontiguousarray(
        w1t.reshape(L, N_IN_CH, 128, D_Z).transpose(0, 2, 1, 3))

    cols = np.zeros((L, N_COLS, D_Z), np.float32)
    for ell in range(L):
        cols[ell, 0] = b1_0 if ell == 0 else b1_r[ell - 1]
        cols[ell, 1] = f(inputs["spec_wr"])[ell, 0]
        cols[ell, 2] = f(inputs["ln_g"])[ell]
        cols[ell, 3] = f(inputs["ln_b"])[ell]
        cols[ell, 4] = f(inputs["Wg"])[ell, 0]
    cols_pc = np.ascontiguousarray(cols.transpose(0, 2, 1))

    common = {
        "w1t": w1t_pc, "xs": xs_pc, "cols": cols_pc,
        "wvt": np.ascontiguousarray(f(inputs["Wv"]).transpose(0, 2, 1)),
        "wkt": np.ascontiguousarray(f(inputs["Wk"]).transpose(0, 2, 1)),
        "bv": f(inputs["bv"]), "bk": f(inputs["bk"]),
        "bg": f(inputs["bg"]).reshape(1, L),
        "decay": f(inputs["decay"]).reshape(1, L),
    }
    in_maps = []
    for c in range(N_CORES):
        sl = slice(c * NSH, (c + 1) * NSH)
        # slot p*NCH + ch within the shard -> [p, ch] (partition-major)
        in_maps.append(dict(
            common,
            m_sh=np.ascontiguousarray(M[:, sl, :]).reshape(L, 128, NCH, D_V),
            k_sh=np.ascontiguousarray(K_mem[:, sl, :]).reshape(L, 128, NCH, D_K)))
    return in_maps


_BUILD_CACHE = {}


def kernel(**inputs):
    _ensure_path()
    from concourse import bass_utils

    if 1 not in _BUILD_CACHE:
        _BUILD_CACHE[1] = build(iters=1)
    nc = _BUILD_CACHE[1]

    in_maps = marshal(inputs)
    r = bass_utils.run_bass_kernel_spmd(nc, in_maps,
                                        core_ids=list(range(N_CORES)))
    # epilogue: the device returned Out = M + (wgt*alpha/keep) x v; apply
    # the per-level keep scale here (exact same math, folded off-device).
    keep = (1.0 - np.asarray(inputs["decay"], np.float32)).reshape(L, 1, 1)
    full = np.empty((L, N_FULL, D_V + D_K), np.float32)
    for c in range(N_CORES):
        full[:, c * NSH:(c + 1) * NSH, :] = \
            r.results[c]["out"].reshape(L, NSH, D_O) * keep
    return full
